# revision 1
# baseline (speedup 1.0000x reference)
"""Trainium2 Bass kernel for per-sample weight-demodulated 3x3 conv + leaky ReLU.

Problem (hardcoded shapes):
  input_vector: (8, 256, 128, 128) f32
  style_vector: (8, 256) f32
  weight:       (256, 256, 3, 3) f32
  out:          (8, 256, 128, 128) f32

Math (faithful to reference):
  ws[b,o,i,kh,kw] = weight[o,i,kh,kw] * style[b,i]
  demod[b,kw]     = rsqrt(sum_{o,i,kh} ws^2 + 1e-8)        # NOTE: sum excludes kw
  y[b] = leaky_relu(conv2d_same(x[b], ws[b]*demod), 0.2)

Sharding: data-parallel over batch, one sample per NeuronCore (8 cores).

Per-core kernel (optimized; TimelineSim ~263 us vs ~290 us baseline):
  - x and w shipped from host as fp16 (halves HBM traffic; conv accumulates in
    f32 PSUM — ~1e-3 rel err vs the 2e-2 gate). y returned fp16, host upcasts.
  - ALL DMAs ride the SP queue (HW showed ACT-queue DMA->PE semaphore edges
    losing races). Order: w first, stile, x pieces 0-2 (issued pre-prep so
    their transfers overlap weight prep), then pieces 3-7 + merged y drains at
    their conv sites.
  - PE warmup: throwaway ident transposes overlap the w-DMA wait, ramping the
    PE clock (HAM) and absorbing the identity-ready semaphore; a dummy sqrt at
    t=0 pulls the ACT function-set table load off the critical path.
  - Weight prep kw-major, big ops: per (kw, kb) the 3 kh x 2 mb [128,128]
    transposes land in one [128,768] f16 psum tile; one ACT activation(Copy,
    scale=style) scales into f32 wfin; ONE ACT Square+accum_out per (kw, kb)
    forms the demod sum of squares; ones-matmul broadcasts the cross-partition
    sum; eps/sqrt/recip; one DVE tensor_scalar rescale+fp16-cast per (kw, kb)
    into wfin16. Each kw's dps-matmul is emitted after the NEXT kw's
    transposes so it never blocks the PE FIFO.
  - x staged in 8 pieces of 18 padded rows (fp16, 130 wide, zero pad via
    memset), each piece in its OWN tile (no buffer reuse: rotating x buffers
    lost write-after-read races on HW).
  - Conv: per 4-row chunk (N=512), 36 fp16 matmuls accumulate into 2 f32 PSUM
    tiles in kw-major tap order, so chunk 0 streams right behind the per-kw
    weight rescales with no PE stall. Epilogue: leaky relu = max(x, 0.2x) on
    DVE, both mb written into
    one [128, 2, N] tile, drained by a single merged y DMA per chunk
    (GPSIMD cannot read PSUM, so DVE does both mb). The last piece ends with
    two 2-row sub-chunks so the drain tail is short.
"""

import numpy as np

B, CIN, COUT, K, H, W = 8, 256, 256, 3, 128, 128
P = 128
KB = CIN // P   # cin partition blocks   = 2
MB = COUT // P  # cout partition blocks  = 2
T = K * K       # taps = 9
WP = W + 2      # padded row width = 130
NP = 8          # x pieces
PROWS = H // NP           # output rows per piece = 16
PPAD = PROWS + 2          # padded rows held per piece = 18
CHUNK_ROWS = 4            # output rows per psum chunk
CHUNK_N = CHUNK_ROWS * W  # matmul free size = 512
CHUNKS_PER_PIECE = PROWS // CHUNK_ROWS  # = 4
N_WARMUP = 40

_CACHE = {}


def _build(stage="full"):
    import concourse.mybir as mybir
    import concourse.tile as tile
    from concourse import bacc
    from concourse.masks import make_identity

    f32 = mybir.dt.float32
    f16 = mybir.dt.float16

    nc = bacc.Bacc(None, target_bir_lowering=False)
    x_d = nc.dram_tensor("x", [CIN, H, W], f16, kind="ExternalInput")
    s_d = nc.dram_tensor("style", [1, CIN], f32, kind="ExternalInput")
    w_d = nc.dram_tensor("w", [COUT, CIN, K, K], f16, kind="ExternalInput")
    y_d = nc.dram_tensor("y", [COUT, H, W], f16, kind="ExternalOutput")

    y_flat = y_d[:].rearrange("o h w -> o (h w)")      # [256, 16384]
    y_pmf = y_d[:].rearrange("(m p) h w -> p m (h w)", p=P)  # [128, 2, 16384]
    w_flat = w_d[:].rearrange("o i kh kw -> o (i kh kw)")  # [256, 2304]

    with tile.TileContext(nc) as tc:
        with (
            tc.tile_pool(name="const", bufs=1) as const,
            tc.tile_pool(name="wtmp", bufs=1) as wtmp,
            tc.tile_pool(name="xbuf", bufs=1) as xbuf,
            tc.tile_pool(name="outp", bufs=3) as outp,
            tc.tile_pool(name="psum", bufs=2, space="PSUM") as psum,
            tc.tile_pool(name="psumw", bufs=2, space="PSUM") as psumw,
            tc.tile_pool(name="psumd", bufs=1, space="PSUM") as psumd,
        ):
            # ---------- constants ----------
            ident = const.tile([P, P], f16)
            make_identity(nc, ident)
            ones = const.tile([P, P], f32)
            nc.vector.memset(ones, 1.0)
            # dummy sqrt so the ACT function-set containing Sqrt loads now,
            # not mid-prep (the table swap drains the ACT pipeline)
            nc.scalar.sqrt(ones[0:1, 0:1], ones[0:1, 0:1])

            # ---------- weight load: very first DMA on the SP queue ----------
            wbuf = wtmp.tile([P, MB, CIN * T], f16)
            nc.sync.dma_start(
                out=wbuf[:],
                in_=w_flat.rearrange("(m p) f -> p m f", p=P),
            )

            # style per-partition: stile[p, kb] = style[kb*128 + p]
            stile = const.tile([P, KB], f32)
            for kb in range(KB):
                nc.sync.dma_start(
                    out=stile[:, kb : kb + 1],
                    in_=s_d[:].rearrange("one c -> c one")[kb * P : (kb + 1) * P, :],
                )

            # ---------- x pieces: alloc + pad + DMA ----------
            xqs = {}

            def stage_piece(p, dma_engine):
                # one tile per piece (no pool-buffer reuse: reused x buffers
                # showed WAR races on HW — sparse halo/edge corruption)
                xq = xbuf.tile([P, KB, PPAD, WP], f16, name=f"xq{p}")
                xqs[p] = xq
                img_lo = p * PROWS - 1
                img_hi = p * PROWS + PROWS  # inclusive
                lo_clip = max(img_lo, 0)
                hi_clip = min(img_hi, H - 1)
                l_lo = lo_clip - img_lo
                nrows = hi_clip - lo_clip + 1
                for kb in range(KB):
                    nc.vector.memset(xq[:, kb, :, 0], 0.0)
                    nc.vector.memset(xq[:, kb, :, WP - 1], 0.0)
                    if img_lo < 0:
                        nc.vector.memset(xq[:, kb, 0, :], 0.0)
                    if img_hi > H - 1:
                        nc.vector.memset(xq[:, kb, PPAD - 1, :], 0.0)
                    dma_engine.dma_start(
                        out=xq[:, kb, l_lo : l_lo + nrows, 1 : 1 + W],
                        in_=x_d[kb * P : (kb + 1) * P, lo_clip : hi_clip + 1, :],
                    )

            # pieces 0-2 issued early so their transfers ride behind w on SP
            for p in range(3):
                stage_piece(p, nc.sync)

            # ---------- PE warmup: ramp the clock while the w DMA flies ------
            for _ in range(N_WARMUP):
                gate = psumw.tile([P, K * MB * P], f16, name="pt")
                nc.tensor.transpose(gate[:, 0:P], ident, ident)

            # ---------- weight prep, kw-major ----------
            # wfin[i_part, kb, t, mb, o] f32 (style-scaled), wfin16 = *demod fp16
            wfin = const.tile([P, KB, T, MB, P], f32)
            wfin16 = const.tile([P, KB, T, MB, P], f16)
            wview = wbuf[:].rearrange("p m (i t) -> p m t i", t=T)  # strided view

            sp = wtmp.tile([P, KB, K], f32)
            spc = wtmp.tile([P, K], f32)
            junk = wtmp.tile([P, K * MB * P], f32)
            dps = psumd.tile([P, K], f32)
            demod = const.tile([P, K], f32)

            def emit_group(kw):
                # transpose the 6 (kh, mb) tiles of each kb into one psum tile,
                # then one style-scale op per kb, then the sum-of-squares
                for kb in range(KB):
                    pt = psumw.tile([P, K * MB * P], f16, name="pt")
                    ptv = pt[:].rearrange("p (kh mb o) -> p kh mb o", kh=K, mb=MB)
                    for kh in range(K):
                        t = kh * K + kw
                        for mb in range(MB):
                            nc.tensor.transpose(
                                ptv[:, kh, mb, :],
                                wview[:, mb, t, kb * P : (kb + 1) * P],
                                ident,
                            )
                    nc.scalar.activation(
                        out=wfin[:, kb, kw::K, :, :],
                        in_=ptv,
                        func=mybir.ActivationFunctionType.Copy,
                        scale=stile[:, kb : kb + 1],
                    )
                # demod[kw] numerator: sum of squares over (o, i, kh) in ONE
                # ACT op per kb: Square(wfin) with free-dim accumulator
                for kb in range(KB):
                    nc.scalar.activation(
                        out=junk[:].rearrange("p (a b c) -> p a b c", a=K, b=MB),
                        in_=wfin[:, kb, kw::K, :, :],
                        func=mybir.ActivationFunctionType.Square,
                        accum_out=sp[:, kb, kw : kw + 1],
                    )
                nc.vector.tensor_add(
                    out=spc[:, kw : kw + 1],
                    in0=sp[:, 0, kw : kw + 1],
                    in1=sp[:, 1, kw : kw + 1],
                )

            def emit_tail(kw):
                # cross-partition sum broadcast to all partitions, demod chain,
                # rescale + fp16 cast (one op per kb)
                nc.tensor.matmul(
                    dps[:, kw : kw + 1], ones, spc[:, kw : kw + 1],
                    start=True, stop=True,
                )
                nc.vector.tensor_scalar_add(
                    demod[:, kw : kw + 1], dps[:, kw : kw + 1], 1e-8
                )
                nc.scalar.sqrt(demod[:, kw : kw + 1], demod[:, kw : kw + 1])
                nc.vector.reciprocal(demod[:, kw : kw + 1], demod[:, kw : kw + 1])
                for kb in range(KB):
                    nc.vector.tensor_scalar_mul(
                        out=wfin16[:, kb, kw::K, :, :],
                        in0=wfin[:, kb, kw::K, :, :],
                        scalar1=demod[:, kw : kw + 1],
                    )

            # software-pipeline: each kw's dps-matmul is emitted after the NEXT
            # kw's transposes so it never blocks them in the PE FIFO
            emit_group(0)
            emit_group(1)
            emit_tail(0)
            emit_group(2)
            emit_tail(1)
            emit_tail(2)

            if stage == "wprep":
                ot = outp.tile([P, KB * T * MB * P], f16)
                nc.vector.tensor_copy(
                    out=ot, in_=wfin16[:].rearrange("p a b c d -> p (a b c d)")
                )
                nc.sync.dma_start(out=y_flat[0:P, 0 : KB * T * MB * P], in_=ot)
                ot2 = outp.tile([P, K], f16)
                nc.vector.tensor_copy(out=ot2, in_=demod)
                nc.sync.dma_start(out=y_flat[0:P, 16000 : 16000 + K], in_=ot2)

            if stage == "full":
                # ---------- conv over 8 pieces ----------
                # last chunk split in two (2-row) so the drain tail is shorter
                for p in range(NP):
                    if p >= 3:
                        stage_piece(p, nc.sync)
                    xq = xqs[p]
                    if p < NP - 1:
                        chunks = [(j * CHUNK_ROWS, CHUNK_ROWS)
                                  for j in range(CHUNKS_PER_PIECE)]
                    else:
                        chunks = [(j * CHUNK_ROWS, CHUNK_ROWS)
                                  for j in range(CHUNKS_PER_PIECE - 1)]
                        half = CHUNK_ROWS // 2
                        base = (CHUNKS_PER_PIECE - 1) * CHUNK_ROWS
                        chunks += [(base, half), (base + half, half)]
                    for lr0, nrows in chunks:
                        r0 = p * PROWS + lr0
                        n_free = nrows * W
                        pts = [
                            psum.tile([P, CHUNK_N], f32, name=f"pc{mb}")
                            for mb in range(MB)
                        ]
                        # kw-major tap order: chunk 0 streams behind the
                        # per-kw weight rescales with no PE stall
                        first = True
                        for kw in range(K):
                            for kb in range(KB):
                                for kh in range(K):
                                    t = kh * K + kw
                                    rhs = xq[
                                        :, kb, lr0 + kh : lr0 + kh + nrows,
                                        kw : kw + W,
                                    ]
                                    last = kw == K - 1 and kb == KB - 1 and kh == K - 1
                                    for mb in range(MB):
                                        nc.tensor.matmul(
                                            pts[mb][:, 0:n_free],
                                            wfin16[:, kb, t, mb, :],
                                            rhs,
                                            start=first,
                                            stop=last,
                                        )
                                    first = False
                        # leaky relu = max(x, 0.2x) on DVE (idle during conv),
                        # fp16 out; y DMA on SP. The short tail sub-chunks
                        # split mb across DVE/GPSIMD so the final drain chain
                        # is half as long.
                        ot = outp.tile([P, MB, CHUNK_N], f16, name="ot")
                        for mb in range(MB):
                            tmp = outp.tile([P, CHUNK_N], f32, name=f"lt{mb}")
                            nc.vector.tensor_scalar_mul(
                                tmp[:, 0:n_free], pts[mb][:, 0:n_free], 0.2
                            )
                            nc.vector.tensor_tensor(
                                out=ot[:, mb, 0:n_free], in0=pts[mb][:, 0:n_free],
                                in1=tmp[:, 0:n_free],
                                op=mybir.AluOpType.max,
                            )
                        nc.sync.dma_start(
                            out=y_pmf[:, :, r0 * W : r0 * W + n_free],
                            in_=ot[:, :, 0:n_free],
                        )
    nc.compile()
    return nc


def _get_nc():
    if "nc" not in _CACHE:
        _CACHE["nc"] = _build()
    return _CACHE["nc"]


def prep_in_maps(input_vector, style_vector, weight):
    """Host-side staging: fp16 casts, per-core input dicts."""
    x16 = np.ascontiguousarray(input_vector, dtype=np.float16)
    w16 = np.ascontiguousarray(weight, dtype=np.float16)
    s32 = np.ascontiguousarray(style_vector, dtype=np.float32)
    return [
        {"x": x16[b], "style": s32[b : b + 1], "w": w16}
        for b in range(B)
    ]


def _get_runner():
    """Build (once) a reusable jitted shard_map runner over the 8 cores, so
    repeated kernel() calls skip re-tracing/lowering the bass module."""
    if "runner" in _CACHE:
        return _CACHE["runner"]

    import jax
    import concourse.bass2jax as b2j
    import concourse.mybir as mybir
    from jax.experimental.shard_map import shard_map
    from jax.sharding import Mesh, PartitionSpec

    nc = _get_nc()
    b2j.install_neuronx_cc_hook()

    partition_name = nc.partition_id_tensor.name if nc.partition_id_tensor else None
    in_names, out_names, out_avals, zero_outs = [], [], [], []
    for alloc in nc.m.functions[0].allocations:
        if not isinstance(alloc, mybir.MemoryLocationSet):
            continue
        name = alloc.memorylocations[0].name
        if alloc.kind == "ExternalInput":
            if name != partition_name:
                in_names.append(name)
        elif alloc.kind == "ExternalOutput":
            out_names.append(name)
            shape = tuple(alloc.tensor_shape)
            dtype = mybir.dt.np(alloc.dtype)
            out_avals.append(jax.core.ShapedArray(shape, dtype))
            zero_outs.append(np.zeros(shape, dtype))
    n_params = len(in_names)
    n_outs = len(out_avals)
    all_in_names = list(in_names) + list(out_names)
    if partition_name is not None:
        all_in_names.append(partition_name)

    def _body(*args):
        operands = list(args)
        if partition_name is not None:
            operands.append(b2j.partition_id_tensor())
        outs = b2j._bass_exec_p.bind(
            *operands,
            out_avals=tuple(out_avals),
            in_names=tuple(all_in_names),
            out_names=tuple(out_names),
            lowering_input_output_aliases=(),
            sim_require_finite=True,
            sim_require_nnan=True,
            nc=nc,
        )
        return tuple(outs)

    devices = jax.devices()[:B]
    mesh = Mesh(np.asarray(devices), ("core",))
    in_specs = (PartitionSpec("core"),) * (n_params + n_outs)
    out_specs = (PartitionSpec("core"),) * len(out_names)
    sharded = jax.jit(
        shard_map(_body, mesh=mesh, in_specs=in_specs, out_specs=out_specs,
                  check_rep=False),
        donate_argnums=tuple(range(n_params, n_params + n_outs)),
        keep_unused=True,
    )
    _CACHE["runner"] = (sharded, in_names, out_names, out_avals, zero_outs)
    return _CACHE["runner"]


def kernel(input_vector, style_vector, weight):
    in_maps = prep_in_maps(input_vector, style_vector, weight)
    try:
        sharded, in_names, out_names, out_avals, zero_outs = _get_runner()
        concat_in = [
            np.concatenate([in_maps[c][nm] for c in range(B)], axis=0)
            for nm in in_names
        ]
        zeros = [
            np.zeros((B * z.shape[0], *z.shape[1:]), z.dtype) for z in zero_outs
        ]
        out_arrs = sharded(*concat_in, *zeros)
        yi = out_names.index("y")
        out = np.asarray(out_arrs[yi]).reshape(B, *out_avals[yi].shape)
    except Exception:
        # fallback: the one-shot path (slower per call, same result)
        from concourse.bass_utils import run_bass_kernel_spmd

        _CACHE.pop("runner", None)
        res = run_bass_kernel_spmd(_get_nc(), in_maps, core_ids=list(range(B)))
        out = np.stack([res.results[b]["y"] for b in range(B)], axis=0)
    return out.astype(np.float32)



# revision 12
# speedup vs baseline: 207.7979x; 207.7979x over previous
"""Trainium2 Bass kernel for per-sample weight-demodulated 3x3 conv + leaky ReLU.

Problem (hardcoded shapes):
  input_vector: (8, 256, 128, 128) f32
  style_vector: (8, 256) f32
  weight:       (256, 256, 3, 3) f32
  out:          (8, 256, 128, 128) f32

Math (faithful to reference):
  ws[b,o,i,kh,kw] = weight[o,i,kh,kw] * style[b,i]
  demod[b,kw]     = rsqrt(sum_{o,i,kh} ws^2 + 1e-8)        # NOTE: sum excludes kw
  y[b] = leaky_relu(conv2d_same(x[b], ws[b]*demod), 0.2)

Sharding: data-parallel over batch, one sample per NeuronCore (8 cores).

Per-core kernel: 1D Winograd F(4,3) along the kw axis (kh stays direct),
which halves the PE matmul work vs the direct conv:
  - weight prep (as in the direct kernel): w DMA'd f16, PE-transposed,
    ACT style-scaled into wfin f16, demod computed per kw via ACT
    Square+accum and a ones-matmul cross-partition broadcast.
  - the Winograd kernel transform G (rows for u=0..5) is folded together
    with demod into per-partition coefficient tiles cu[u,kw] =
    64*G[u,kw]*demod[kw]; U[u,kh] = sum_kw cu[u,kw]*wfin[kh,kw] built by
    ACT copy-scale + DVE scalar_tensor_tensor chains, stored f16.  The
    x64 scale keeps U out of the f16-denormal range; it is compensated
    exactly by scale=1/64 inside the final ACT Prelu.
  - input transform on DVE: 3 contiguous helper ops per (piece, cin-half)
    (A=x_c-x_{c+2}, T=x_{c+1}+x_{c+2}, D=x_{c+1}-x_{c+2}; fp16, packed)
    then 6 fused scalar_tensor_tensor ops produce d[u] (d3/d4 on GPSIMD).
  - conv: per 8-row chunk and cout-half, 36 fp16 matmuls (6 u-comps x 3 kh
    x 2 cin-halves, free = 8 rows x 32 tiles) accumulate into one
    [128,6,8,32] f32 PSUM tile (3 banks, double-buffered).
  - epilogue: ACT copies m1..m4 from PSUM to SBUF, DVE applies the A^T
    combine with fused STT ops into ytmp f32 (strided j::4 writes), ACT
    applies leaky-ReLU natively (parametric_relu alpha=0.2, scale=1/64)
    casting to f16, one merged y DMA per chunk on the SP queue.
  - x is staged in 8 pieces of 18 padded rows, each in its OWN tile (the
    direct kernel showed DMA write-after-read races with rotating x
    buffers on HW); all DMAs ride the SP queue; PE warmup transposes ramp
    the clock while the w DMA flies; a dummy sqrt at t=0 pulls the ACT
    function-set load off the critical path (copy/square/sqrt/
    parametric_relu share one set).
"""

import numpy as np

B, CIN, COUT, K, H, W = 8, 256, 256, 3, 128, 128
P = 128
KB = CIN // P   # cin partition blocks   = 2
MB = COUT // P  # cout partition blocks  = 2
T = K * K       # taps = 9
NU = 6          # winograd F(4,3) components
NT = W // 4     # winograd tiles per row = 32
WP = W + 2      # padded row width = 130
NP = 8          # x pieces
PROWS = H // NP           # output rows per piece = 16
PPAD = PROWS + 2          # padded rows held per piece = 18
CH_ROWS = 8               # output rows per psum chunk
CHUNKS = PROWS // CH_ROWS  # = 2
N_WARMUP = 40
USCALE = 64.0

# F(4,3) kernel transform G (Lavin), rows u=0..5 over kw=0..2
G_ROWS = [
    [1.0 / 4.0, 0.0, 0.0],
    [-1.0 / 6.0, -1.0 / 6.0, -1.0 / 6.0],
    [-1.0 / 6.0, 1.0 / 6.0, -1.0 / 6.0],
    [1.0 / 24.0, 1.0 / 12.0, 1.0 / 6.0],
    [1.0 / 24.0, -1.0 / 12.0, 1.0 / 6.0],
    [0.0, 0.0, 1.0],
]

_CACHE = {}


def _build(stage="full"):
    import concourse.mybir as mybir
    import concourse.tile as tile
    from concourse import bacc
    from concourse.masks import make_identity

    f32 = mybir.dt.float32
    f16 = mybir.dt.float16
    Alu = mybir.AluOpType
    Act = mybir.ActivationFunctionType

    nc = bacc.Bacc(None, target_bir_lowering=False)
    x_d = nc.dram_tensor("x", [CIN, H, W], f16, kind="ExternalInput")
    s_d = nc.dram_tensor("style", [1, CIN], f32, kind="ExternalInput")
    w_d = nc.dram_tensor("w", [COUT, CIN, K, K], f16, kind="ExternalInput")
    y_d = nc.dram_tensor("y", [COUT, H, W], f16, kind="ExternalOutput")

    y_flat = y_d[:].rearrange("o h w -> o (h w)")      # [256, 16384]
    y_pmf = y_d[:].rearrange("(m p) h w -> p m (h w)", p=P)  # [128, 2, 16384]
    w_flat = w_d[:].rearrange("o i kh kw -> o (i kh kw)")  # [256, 2304]

    with tile.TileContext(nc) as tc:
        with (
            tc.tile_pool(name="const", bufs=1) as const,
            tc.tile_pool(name="wtmp", bufs=1) as wtmp,
            tc.tile_pool(name="utmp", bufs=2) as utmp,
            tc.tile_pool(name="xbuf", bufs=1) as xbuf,
            tc.tile_pool(name="dbuf", bufs=2) as dbuf,
            tc.tile_pool(name="ttmp", bufs=2) as ttmp,
            tc.tile_pool(name="ctmp", bufs=2) as ctmp,
            tc.tile_pool(name="outp", bufs=2) as outp,
            tc.tile_pool(name="psum", bufs=2, space="PSUM") as psum,
            tc.tile_pool(name="psumw", bufs=1, space="PSUM") as psumw,
            tc.tile_pool(name="psumd", bufs=1, space="PSUM") as psumd,
        ):
            # ---------- constants ----------
            ident = const.tile([P, P], f16)
            make_identity(nc, ident)
            ones = const.tile([P, P], f32)
            nc.vector.memset(ones, 1.0)
            # dummy sqrt so the ACT function-set containing Sqrt (and Copy/
            # Square/parametric_relu) loads now, not mid-prep
            nc.scalar.sqrt(ones[0:1, 0:1], ones[0:1, 0:1])

            # ---------- weight load: very first DMA on the SP queue ----------
            wbuf = wtmp.tile([P, MB, CIN * T], f16)
            nc.sync.dma_start(
                out=wbuf[:],
                in_=w_flat.rearrange("(m p) f -> p m f", p=P),
            )

            # style per-partition: stile[p, kb] = style[kb*128 + p]
            stile = const.tile([P, KB], f32)
            for kb in range(KB):
                nc.sync.dma_start(
                    out=stile[:, kb : kb + 1],
                    in_=s_d[:].rearrange("one c -> c one")[kb * P : (kb + 1) * P, :],
                )

            # ---------- x: one big padded tile, disjoint piece sub-DMAs ------
            # single tile written exactly once (disjoint regions) -> no
            # buffer-reuse WAR hazard; sub-DMAs preserve transfer/compute
            # overlap.  Padded rows 0..129 hold image rows -1..128.
            xq = xbuf.tile([P, KB, H + 2, WP], f16, name="xq")
            for kb in range(KB):
                nc.vector.memset(xq[:, kb, :, 0], 0.0)
                nc.vector.memset(xq[:, kb, :, WP - 1], 0.0)
                nc.vector.memset(xq[:, kb, 0, :], 0.0)
                nc.vector.memset(xq[:, kb, H + 1, :], 0.0)
            for p in range(NP):
                r_lo = p * PROWS
                for kb in range(KB):
                    nc.sync.dma_start(
                        out=xq[:, kb, r_lo + 1 : r_lo + 1 + PROWS, 1 : 1 + W],
                        in_=x_d[kb * P : (kb + 1) * P, r_lo : r_lo + PROWS, :],
                    )

            # ---------- PE warmup: ramp the clock while the w DMA flies ------
            for _ in range(N_WARMUP):
                gate = psumw.tile([P, K * MB * P], f16, name="pt")
                nc.tensor.transpose(gate[:, 0:P], ident, ident)

            # ---------- weight prep, kw-major ----------
            # wfin[i_part, kb, t, mb, o] f16 (style-scaled)
            wfin = const.tile([P, KB, T, MB, P], f16)
            wview = wbuf[:].rearrange("p m (i t) -> p m t i", t=T)  # strided view

            sp = wtmp.tile([P, KB, K], f32)
            spc = wtmp.tile([P, K], f32)
            junk = wtmp.tile([P, K * MB * P], f16)
            dps = psumd.tile([P, K], f32)
            demod = const.tile([P, K], f32)

            def emit_group(kw):
                # transpose the 6 (kh, mb) tiles of each kb into one psum tile,
                # then one style-scale op per kb, then the sum-of-squares
                for kb in range(KB):
                    pt = psumw.tile([P, K * MB * P], f16, name="pt")
                    ptv = pt[:].rearrange("p (kh mb o) -> p kh mb o", kh=K, mb=MB)
                    for kh in range(K):
                        t = kh * K + kw
                        for mb in range(MB):
                            nc.tensor.transpose(
                                ptv[:, kh, mb, :],
                                wview[:, mb, t, kb * P : (kb + 1) * P],
                                ident,
                            )
                    nc.scalar.activation(
                        out=wfin[:, kb, kw::K, :, :],
                        in_=ptv,
                        func=Act.Copy,
                        scale=stile[:, kb : kb + 1],
                    )
                # demod[kw] numerator: sum of squares over (o, i, kh) in ONE
                # ACT op per kb: Square(wfin) with free-dim accumulator
                for kb in range(KB):
                    nc.scalar.activation(
                        out=junk[:].rearrange("p (a b c) -> p a b c", a=K, b=MB),
                        in_=wfin[:, kb, kw::K, :, :],
                        func=Act.Square,
                        accum_out=sp[:, kb, kw : kw + 1],
                    )
                nc.vector.tensor_add(
                    out=spc[:, kw : kw + 1],
                    in0=sp[:, 0, kw : kw + 1],
                    in1=sp[:, 1, kw : kw + 1],
                )

            def emit_tail(kw):
                # cross-partition sum broadcast to all partitions, demod chain
                nc.tensor.matmul(
                    dps[:, kw : kw + 1], ones, spc[:, kw : kw + 1],
                    start=True, stop=True,
                )
                nc.vector.tensor_scalar_add(
                    demod[:, kw : kw + 1], dps[:, kw : kw + 1], 1e-8
                )
                nc.scalar.sqrt(demod[:, kw : kw + 1], demod[:, kw : kw + 1])
                nc.vector.reciprocal(demod[:, kw : kw + 1], demod[:, kw : kw + 1])

            # software-pipeline: each kw's dps-matmul is emitted after the NEXT
            # kw's transposes so it never blocks them in the PE FIFO
            emit_group(0)
            emit_group(1)
            emit_tail(0)
            emit_group(2)
            emit_tail(1)
            emit_tail(2)

            # ---------- winograd coefficient tiles cu[u,kw] = 64*G*demod -----
            # u=3's d-component is computed NEGATED on gpsimd (plain add; the
            # Pool engine lacks scalar_tensor_tensor), compensated by negating
            # its weight coefficients here — exact.
            cu = const.tile([P, NU, K], f32)
            for u in range(NU):
                for kw in range(K):
                    g = G_ROWS[u][kw]
                    if g != 0.0:
                        sign = -1.0 if u == 3 else 1.0
                        nc.vector.tensor_scalar_mul(
                            out=cu[:, u, kw : kw + 1],
                            in0=demod[:, kw : kw + 1],
                            scalar1=float(USCALE * g * sign),
                        )

            # ---------- winograd-transformed weights Ub[u] -------------------
            # Ub[i_part, kb, kh, u, mb, o] f16 = sum_kw cu[u,kw]*wfin[kh,kw]
            Ub = const.tile([P, KB, K, NU, MB, P], f16)

            def emit_U(u):
                nz = [kw for kw in range(K) if G_ROWS[u][kw] != 0.0]
                for kb in range(KB):
                    for kh in range(K):
                        for mb in range(MB):
                            srcs = [
                                wfin[:, kb, kh * K + kw, mb, :] for kw in nz
                            ]
                            dst = Ub[:, kb, kh, u, mb, :]
                            if len(nz) == 1:
                                nc.scalar.activation(
                                    out=dst, in_=srcs[0], func=Act.Copy,
                                    scale=cu[:, u, nz[0] : nz[0] + 1],
                                )
                            else:
                                ta = utmp.tile([P, P], f32, name="ta")
                                nc.scalar.activation(
                                    out=ta, in_=srcs[0], func=Act.Copy,
                                    scale=cu[:, u, nz[0] : nz[0] + 1],
                                )
                                tb = utmp.tile([P, P], f32, name="tb")
                                nc.vector.scalar_tensor_tensor(
                                    out=tb, in0=srcs[1],
                                    scalar=cu[:, u, nz[1] : nz[1] + 1],
                                    in1=ta, op0=Alu.mult, op1=Alu.add,
                                )
                                nc.vector.scalar_tensor_tensor(
                                    out=dst, in0=srcs[2],
                                    scalar=cu[:, u, nz[2] : nz[2] + 1],
                                    in1=tb, op0=Alu.mult, op1=Alu.add,
                                )

            # ---------- input transform ----------
            dqs = {}

            def ensure_dq(p):
                if p not in dqs:
                    dqs[p] = dbuf.tile([P, KB, PPAD, NU, NT], f16, name="dq")
                return dqs[p]

            def emit_helpers(p, kb):
                xv = xq[:, kb, p * PROWS : p * PROWS + PPAD, :]  # [P, PPAD, WP]
                A = ttmp.tile([P, PPAD, W], f16, name="A")
                TT = ttmp.tile([P, PPAD, W], f16, name="T")
                DD = ttmp.tile([P, PPAD, W], f16, name="D")
                nc.vector.tensor_sub(A, xv[:, :, 0:W], xv[:, :, 2 : W + 2])
                nc.vector.tensor_add(TT, xv[:, :, 1 : W + 1], xv[:, :, 2 : W + 2])
                nc.vector.tensor_sub(DD, xv[:, :, 1 : W + 1], xv[:, :, 2 : W + 2])
                return A, TT, DD

            def emit_d(p, kb, helpers, u):
                A, TT, DD = helpers
                out = ensure_dq(p)[:, kb, :, u, :]
                if u == 0:
                    nc.vector.scalar_tensor_tensor(
                        out=out, in0=A[:, :, 0::4], scalar=4.0,
                        in1=A[:, :, 2::4], op0=Alu.mult, op1=Alu.subtract)
                elif u == 1:
                    nc.vector.scalar_tensor_tensor(
                        out=out, in0=TT[:, :, 0::4], scalar=-4.0,
                        in1=TT[:, :, 2::4], op0=Alu.mult, op1=Alu.add)
                elif u == 2:
                    nc.vector.scalar_tensor_tensor(
                        out=out, in0=DD[:, :, 0::4], scalar=4.0,
                        in1=DD[:, :, 2::4], op0=Alu.mult, op1=Alu.subtract)
                elif u == 3:
                    # gpsimd path (Pool has no scalar_tensor_tensor):
                    # h = 2*A1 (add), d3n = h + A2 = -d3 (sign folded into
                    # cu[3]), d4 = h - A2.  Emits BOTH u=3 and u=4; the u=4
                    # call is a no-op.
                    out4 = ensure_dq(p)[:, kb, :, 4, :]
                    h = ttmp.tile([P, PPAD, NT], f16, name="h2")
                    nc.gpsimd.tensor_add(h, A[:, :, 1::4], A[:, :, 1::4])
                    nc.gpsimd.tensor_add(out, h, A[:, :, 2::4])
                    nc.gpsimd.tensor_sub(out4, h, A[:, :, 2::4])
                elif u == 4:
                    pass
                elif u == 5:
                    nc.vector.scalar_tensor_tensor(
                        out=out, in0=A[:, :, 1::4], scalar=4.0,
                        in1=A[:, :, 3::4], op0=Alu.mult, op1=Alu.subtract)

            def emit_transform(p, kb):
                h = emit_helpers(p, kb)
                for u in range(NU):
                    emit_d(p, kb, h, u)

            # piece-0 transform woven with the U build so the PE's first
            # chunk (which consumes u-components in order) starts early
            h00 = emit_helpers(0, 0)
            h01 = emit_helpers(0, 1)
            emit_d(0, 0, h00, 0)
            emit_d(0, 1, h01, 0)
            emit_U(0)
            for u in range(1, NU):
                emit_U(u)
                emit_d(0, 0, h00, u)
                emit_d(0, 1, h01, u)

            if stage == "wprep":
                ot = outp.tile([P, KB * K * NU * MB * P], f16)
                nc.vector.tensor_copy(
                    out=ot, in_=Ub[:].rearrange("p a b c d e -> p (a b c d e)")
                )
                nc.sync.dma_start(out=y_flat[0:P, 0 : KB * K * NU * MB * P], in_=ot)
                ot2 = outp.tile([P, K], f16)
                nc.vector.tensor_copy(out=ot2, in_=demod)
                nc.sync.dma_start(out=y_flat[0:P, 16000 : 16000 + K], in_=ot2)

            if stage == "dtrans":
                dq0 = dqs[0]
                ot = outp.tile([P, KB * PPAD * NU * NT], f16)
                nc.vector.tensor_copy(
                    out=ot, in_=dq0[:].rearrange("p a b c d -> p (a b c d)")
                )
                nc.sync.dma_start(
                    out=y_flat[0:P, 0 : KB * PPAD * NU * NT], in_=ot
                )

            if stage == "full":
                # ---------- conv over 8 pieces ----------
                def emit_chunk(p, c):
                    dq = dqs[p]
                    lr0 = c * CH_ROWS
                    r0 = p * PROWS + lr0
                    yt = outp.tile([P, MB, CH_ROWS, W], f32, name="yt")
                    ots = outp.tile([P, MB, CH_ROWS, W], f16, name="ot")
                    for mb in range(MB):
                        pt = psum.tile([P, NU, CH_ROWS, NT], f32, name="pc")
                        for u in range(NU):
                            first = True
                            for kb in range(KB):
                                for kh in range(K):
                                    nc.tensor.matmul(
                                        pt[:, u],
                                        Ub[:, kb, kh, u, mb, :],
                                        dq[:, kb, lr0 + kh : lr0 + kh + CH_ROWS, u, :],
                                        start=first,
                                        stop=(kb == KB - 1 and kh == K - 1),
                                    )
                                    first = False
                        # ---- epilogue: A^T combine ----
                        cs = []
                        for i, mi in enumerate((1, 2, 3, 4)):
                            cm = ctmp.tile([P, CH_ROWS, NT], f32, name=f"cm{i}")
                            nc.scalar.activation(
                                out=cm, in_=pt[:, mi], func=Act.Copy)
                            cs.append(cm)
                        pp = ctmp.tile([P, CH_ROWS, NT], f32, name="pp")
                        qq = ctmp.tile([P, CH_ROWS, NT], f32, name="qq")
                        rr = ctmp.tile([P, CH_ROWS, NT], f32, name="rr")
                        ss = ctmp.tile([P, CH_ROWS, NT], f32, name="ss")
                        nc.vector.tensor_add(pp, cs[0], cs[1])
                        nc.vector.tensor_sub(qq, cs[0], cs[1])
                        nc.vector.tensor_add(rr, cs[2], cs[3])
                        nc.vector.tensor_sub(ss, cs[2], cs[3])
                        tt0 = ctmp.tile([P, CH_ROWS, NT], f32, name="tt0")
                        nc.vector.tensor_add(tt0, pt[:, 0], pp)
                        ytv = yt[:, mb]
                        nc.vector.tensor_add(ytv[:, :, 0::4], tt0, rr)
                        nc.vector.scalar_tensor_tensor(
                            out=ytv[:, :, 1::4], in0=ss, scalar=2.0, in1=qq,
                            op0=Alu.mult, op1=Alu.add)
                        nc.vector.scalar_tensor_tensor(
                            out=ytv[:, :, 2::4], in0=rr, scalar=4.0, in1=pp,
                            op0=Alu.mult, op1=Alu.add)
                        y3a = ctmp.tile([P, CH_ROWS, NT], f32, name="y3a")
                        nc.vector.scalar_tensor_tensor(
                            out=y3a, in0=ss, scalar=8.0, in1=qq,
                            op0=Alu.mult, op1=Alu.add)
                        nc.vector.tensor_add(ytv[:, :, 3::4], y3a, pt[:, 5])
                        # leaky relu + 1/64 descale + f16 cast on ACT
                        nc.scalar.activation(
                            out=ots[:, mb], in_=yt[:, mb], func=Act.Prelu,
                            scale=1.0 / USCALE, alpha=0.2)
                    nc.sync.dma_start(
                        out=y_pmf[:, :, r0 * W : r0 * W + CH_ROWS * W],
                        in_=ots,
                    )

                for p in range(NP):
                    emit_chunk(p, 0)
                    if p + 1 < NP:
                        emit_transform(p + 1, 0)
                    emit_chunk(p, 1)
                    if p + 1 < NP:
                        emit_transform(p + 1, 1)
    nc.compile()
    return nc


def _get_nc():
    if "nc" not in _CACHE:
        _CACHE["nc"] = _build()
    return _CACHE["nc"]


def prep_in_maps(input_vector, style_vector, weight):
    """Host-side staging: fp16 casts, per-core input dicts."""
    x16 = np.ascontiguousarray(input_vector, dtype=np.float16)
    w16 = np.ascontiguousarray(weight, dtype=np.float16)
    s32 = np.ascontiguousarray(style_vector, dtype=np.float32)
    return [
        {"x": x16[b], "style": s32[b : b + 1], "w": w16}
        for b in range(B)
    ]


def _get_runner():
    """Build (once) a reusable jitted shard_map runner over the 8 cores, so
    repeated kernel() calls skip re-tracing/lowering the bass module."""
    if "runner" in _CACHE:
        return _CACHE["runner"]

    import jax
    import concourse.bass2jax as b2j
    import concourse.mybir as mybir
    from jax.experimental.shard_map import shard_map
    from jax.sharding import Mesh, PartitionSpec

    nc = _get_nc()
    b2j.install_neuronx_cc_hook()

    partition_name = nc.partition_id_tensor.name if nc.partition_id_tensor else None
    in_names, out_names, out_avals, zero_outs = [], [], [], []
    for alloc in nc.m.functions[0].allocations:
        if not isinstance(alloc, mybir.MemoryLocationSet):
            continue
        name = alloc.memorylocations[0].name
        if alloc.kind == "ExternalInput":
            if name != partition_name:
                in_names.append(name)
        elif alloc.kind == "ExternalOutput":
            out_names.append(name)
            shape = tuple(alloc.tensor_shape)
            dtype = mybir.dt.np(alloc.dtype)
            out_avals.append(jax.core.ShapedArray(shape, dtype))
            zero_outs.append(np.zeros(shape, dtype))
    n_params = len(in_names)
    n_outs = len(out_avals)
    all_in_names = list(in_names) + list(out_names)
    if partition_name is not None:
        all_in_names.append(partition_name)

    def _body(*args):
        operands = list(args)
        if partition_name is not None:
            operands.append(b2j.partition_id_tensor())
        outs = b2j._bass_exec_p.bind(
            *operands,
            out_avals=tuple(out_avals),
            in_names=tuple(all_in_names),
            out_names=tuple(out_names),
            lowering_input_output_aliases=(),
            sim_require_finite=True,
            sim_require_nnan=True,
            nc=nc,
        )
        return tuple(outs)

    devices = jax.devices()[:B]
    mesh = Mesh(np.asarray(devices), ("core",))
    in_specs = (PartitionSpec("core"),) * (n_params + n_outs)
    out_specs = (PartitionSpec("core"),) * len(out_names)
    sharded = jax.jit(
        shard_map(_body, mesh=mesh, in_specs=in_specs, out_specs=out_specs,
                  check_rep=False),
        donate_argnums=tuple(range(n_params, n_params + n_outs)),
        keep_unused=True,
    )
    _CACHE["runner"] = (sharded, in_names, out_names, out_avals, zero_outs)
    return _CACHE["runner"]


def finish_out(y_stack):
    """Raw stacked per-core outputs (B, COUT, H, W) -> full f32 output."""
    return np.ascontiguousarray(y_stack).astype(np.float32)


def kernel(input_vector, style_vector, weight):
    in_maps = prep_in_maps(input_vector, style_vector, weight)
    try:
        sharded, in_names, out_names, out_avals, zero_outs = _get_runner()
        concat_in = [
            np.concatenate([in_maps[c][nm] for c in range(B)], axis=0)
            for nm in in_names
        ]
        zeros = [
            np.zeros((B * z.shape[0], *z.shape[1:]), z.dtype) for z in zero_outs
        ]
        out_arrs = sharded(*concat_in, *zeros)
        yi = out_names.index("y")
        out = np.asarray(out_arrs[yi]).reshape(B, *out_avals[yi].shape)
    except Exception:
        # fallback: the one-shot path (slower per call, same result)
        from concourse.bass_utils import run_bass_kernel_spmd

        _CACHE.pop("runner", None)
        res = run_bass_kernel_spmd(_get_nc(), in_maps, core_ids=list(range(B)))
        out = np.stack([res.results[b]["y"] for b in range(B)], axis=0)
    return out.astype(np.float32)


# revision 16
# speedup vs baseline: 259.5190x; 1.2489x over previous
"""Trainium2 Bass kernel for per-sample weight-demodulated 3x3 conv + leaky ReLU.

Problem (hardcoded shapes):
  input_vector: (8, 256, 128, 128) f32
  style_vector: (8, 256) f32
  weight:       (256, 256, 3, 3) f32
  out:          (8, 256, 128, 128) f32

Math (faithful to reference):
  ws[b,o,i,kh,kw] = weight[o,i,kh,kw] * style[b,i]
  demod[b,kw]     = rsqrt(sum_{o,i,kh} ws^2 + 1e-8)        # NOTE: sum excludes kw
  y[b] = leaky_relu(conv2d_same(x[b], ws[b]*demod), 0.2)

Sharding: data-parallel over batch, one sample per NeuronCore (8 cores).

Per-core kernel: 1D Winograd F(2,3) along the kw axis (kh stays direct),
cutting PE matmul work 1.5x vs the direct conv while keeping the PE the
dominant engine (the HAM clock manager holds full clock only under a
sustained-busy engine; a balanced multi-engine design oscillates at k=4).

  - weight prep: w DMA'd f16, PE-transposed, ACT style-scaled into wfin
    f16, demod per kw via ACT Square+accum and a ones-matmul broadcast.
  - the F(2,3) kernel transform G = [[1,0,0],[.5,.5,.5],[.5,-.5,.5],
    [0,0,1]] folds with demod into per-partition coefficients
    cu[u,kw] = 64*G*demod (u=2 negated, see below); U[u,kh] built by ACT
    copy-scale + DVE scalar_tensor_tensor chains, f16.  The x64 scale
    keeps U out of f16-denormal range; compensated exactly by scale=1/64
    inside the final ACT Prelu.
  - input transform: the F(2,3) data components are EXACTLY strided views
    of three contiguous helper tensors per (piece, cin-half):
        A_c = x_c - x_{c+2}   -> d0 = A[0::2],  d3 = A[1::2]
        T_c = x_{c+1}+x_{c+2} -> d1 = T[0::2]
        D_c = x_{c+1}-x_{c+2} -> d2 = -D[0::2]  (sign folded into cu[2])
    so DVE does just 3 packed-f16 tensor ops per (piece, cin-half) and the
    PE matmuls read the strided views directly.
  - conv: per 8-row chunk and cout-half, 24 f16 matmuls (4 u-comps x 3 kh
    x 2 cin-halves, free = 8 rows x 64 tiles = 512) accumulate into one
    [128,4,8,64] f32 PSUM tile (4 banks, double-buffered = all 8; the PE
    warmup ramp and the demod broadcast borrow tiles from the same pool).
  - epilogue: ACT copies m1,m2 from PSUM; DVE: y[2t] = (m1+m2)+m0,
    y[2t+1] = (m1-m2)-m3 (4 ops); ACT applies leaky-ReLU natively
    (parametric_relu alpha=0.2, scale=1/64, same ACT function set as
    Copy/Square/Sqrt - no table swaps) casting f16; one merged y DMA per
    chunk on the SP queue.
  - x lives in ONE big padded SBUF tile written once by disjoint per-piece
    sub-DMAs (no buffer reuse -> no DMA WAR races; overlap preserved).
"""

import numpy as np

B, CIN, COUT, K, H, W = 8, 256, 256, 3, 128, 128
P = 128
KB = CIN // P   # cin partition blocks   = 2
MB = COUT // P  # cout partition blocks  = 2
T = K * K       # taps = 9
NU = 4          # winograd F(2,3) components
NT = W // 2     # winograd tiles per row = 64
WP = W + 2      # padded row width = 130
NP = 8          # x pieces
PROWS = H // NP           # output rows per piece = 16
PPAD = PROWS + 2          # padded rows per piece = 18
CH_ROWS = 8               # output rows per psum chunk
CHUNKS = PROWS // CH_ROWS  # = 2
N_WARMUP = 40
USCALE = 64.0

# F(2,3) kernel transform G, rows u=0..3 over kw=0..2; u=2 sign-flipped
# because its data component is read as D = -(d2 view).
G_ROWS = [
    [1.0, 0.0, 0.0],
    [0.5, 0.5, 0.5],
    [0.5, -0.5, 0.5],
    [0.0, 0.0, 1.0],
]
U_SIGN = [1.0, 1.0, -1.0, 1.0]

_CACHE = {}


def _build(stage="full"):
    import concourse.mybir as mybir
    import concourse.tile as tile
    from concourse import bacc
    from concourse.masks import make_identity

    f32 = mybir.dt.float32
    f16 = mybir.dt.float16
    Alu = mybir.AluOpType
    Act = mybir.ActivationFunctionType

    nc = bacc.Bacc(None, target_bir_lowering=False)
    x_d = nc.dram_tensor("x", [CIN, H, W], f16, kind="ExternalInput")
    s_d = nc.dram_tensor("style", [1, CIN], f32, kind="ExternalInput")
    w_d = nc.dram_tensor("w", [COUT, CIN, K, K], f16, kind="ExternalInput")
    y_d = nc.dram_tensor("y", [COUT, H, W], f16, kind="ExternalOutput")

    y_flat = y_d[:].rearrange("o h w -> o (h w)")      # [256, 16384]
    y_pmf = y_d[:].rearrange("(m p) h w -> p m (h w)", p=P)  # [128, 2, 16384]
    w_flat = w_d[:].rearrange("o i kh kw -> o (i kh kw)")  # [256, 2304]

    with tile.TileContext(nc) as tc:
        with (
            tc.tile_pool(name="const", bufs=1) as const,
            tc.tile_pool(name="wtmp", bufs=1) as wtmp,
            tc.tile_pool(name="utmp", bufs=2) as utmp,
            tc.tile_pool(name="xbuf", bufs=1) as xbuf,
            tc.tile_pool(name="ttmp", bufs=2) as ttmp,
            tc.tile_pool(name="ctmp", bufs=2) as ctmp,
            tc.tile_pool(name="outp", bufs=2) as outp,
            tc.tile_pool(name="psum", bufs=2, space="PSUM") as psum,
        ):
            # ---------- constants ----------
            ident = const.tile([P, P], f16)
            make_identity(nc, ident)
            ones = const.tile([P, P], f32)
            nc.vector.memset(ones, 1.0)
            # dummy sqrt so the ACT function-set containing Sqrt (and Copy/
            # Square/parametric_relu) loads now, not mid-prep
            nc.scalar.sqrt(ones[0:1, 0:1], ones[0:1, 0:1])

            # ---------- weight load: very first DMA on the SP queue ----------
            wbuf = wtmp.tile([P, MB, CIN * T], f16)
            nc.sync.dma_start(
                out=wbuf[:],
                in_=w_flat.rearrange("(m p) f -> p m f", p=P),
            )

            # style per-partition: stile[p, kb] = style[kb*128 + p]
            stile = const.tile([P, KB], f32)
            for kb in range(KB):
                nc.sync.dma_start(
                    out=stile[:, kb : kb + 1],
                    in_=s_d[:].rearrange("one c -> c one")[kb * P : (kb + 1) * P, :],
                )

            # ---------- x: one big padded tile, disjoint piece sub-DMAs ------
            xq = xbuf.tile([P, KB, H + 2, WP], f16, name="xq")
            for kb in range(KB):
                nc.vector.memset(xq[:, kb, :, 0], 0.0)
                nc.vector.memset(xq[:, kb, :, WP - 1], 0.0)
                nc.vector.memset(xq[:, kb, 0, :], 0.0)
                nc.vector.memset(xq[:, kb, H + 1, :], 0.0)
            for p in range(NP):
                r_lo = p * PROWS
                for kb in range(KB):
                    nc.sync.dma_start(
                        out=xq[:, kb, r_lo + 1 : r_lo + 1 + PROWS, 1 : 1 + W],
                        in_=x_d[kb * P : (kb + 1) * P, r_lo : r_lo + PROWS, :],
                    )

            # ---------- PE warmup: ramp the clock while the w DMA flies ------
            # (borrows conv-psum tiles: matmul of ident into the f32 tile)
            for _ in range(N_WARMUP):
                gate = psum.tile([P, NU, CH_ROWS, NT], f32, name="pc")
                nc.tensor.matmul(
                    gate[:, 0, 0:2, :], ident, ident, start=True, stop=True
                )

            # ---------- weight prep, kw-major ----------
            wfin = const.tile([P, KB, T, MB, P], f16)
            wview = wbuf[:].rearrange("p m (i t) -> p m t i", t=T)  # strided view

            sp = wtmp.tile([P, KB, K], f32)
            spc = wtmp.tile([P, K], f32)
            junk = wtmp.tile([P, K * MB * P], f16)
            demod = const.tile([P, K], f32)
            dps = None  # psum slice for the cross-partition demod broadcast

            def emit_group(kw):
                # transpose the 6 (kh, mb) tiles of each kb into one psum tile,
                # then one style-scale op per kb, then the sum-of-squares
                for kb in range(KB):
                    pt = psum.tile([P, NU, CH_ROWS, NT], f32, name="pc")
                    ptf = pt[:].rearrange("p a b c -> p (a b c)")
                    ptv = ptf[:, 0 : K * MB * P].rearrange(
                        "p (kh mb o) -> p kh mb o", kh=K, mb=MB
                    )
                    # weight "transpose" via ident-matmul (w.T @ I = w^T) so
                    # the result lands in the f32 conv-psum tile directly
                    for kh in range(K):
                        t = kh * K + kw
                        for mb in range(MB):
                            nc.tensor.matmul(
                                ptv[:, kh, mb, :],
                                wview[:, mb, t, kb * P : (kb + 1) * P],
                                ident,
                                start=True, stop=True,
                            )
                    nc.scalar.activation(
                        out=wfin[:, kb, kw::K, :, :],
                        in_=ptv,
                        func=Act.Copy,
                        scale=stile[:, kb : kb + 1],
                    )
                # demod[kw] numerator: sum of squares over (o, i, kh) in ONE
                # ACT op per kb: Square(wfin) with free-dim accumulator
                for kb in range(KB):
                    nc.scalar.activation(
                        out=junk[:].rearrange("p (a b c) -> p a b c", a=K, b=MB),
                        in_=wfin[:, kb, kw::K, :, :],
                        func=Act.Square,
                        accum_out=sp[:, kb, kw : kw + 1],
                    )
                nc.vector.tensor_add(
                    out=spc[:, kw : kw + 1],
                    in0=sp[:, 0, kw : kw + 1],
                    in1=sp[:, 1, kw : kw + 1],
                )

            def emit_tail(kw):
                # cross-partition sum broadcast to all partitions, demod chain
                nc.tensor.matmul(
                    dps[:, kw : kw + 1], ones, spc[:, kw : kw + 1],
                    start=True, stop=True,
                )
                nc.vector.tensor_scalar_add(
                    demod[:, kw : kw + 1], dps[:, kw : kw + 1], 1e-8
                )
                nc.scalar.sqrt(demod[:, kw : kw + 1], demod[:, kw : kw + 1])
                nc.vector.reciprocal(demod[:, kw : kw + 1], demod[:, kw : kw + 1])

            # all groups first; the dps tile is then allocated LAST from the
            # rotating psum pool so no later wprep allocation reclaims its
            # buffer while the demod tails still read it
            emit_group(0)
            emit_group(1)
            emit_group(2)
            dpst = psum.tile([P, NU, CH_ROWS, NT], f32, name="pc")
            dps = dpst[:, 0, 0, 0:K]
            emit_tail(0)
            emit_tail(1)
            emit_tail(2)

            # ---------- input-transform helpers ----------
            helpers = {}

            def emit_helpers(p, kb):
                xv = xq[:, kb, p * PROWS : p * PROWS + PPAD, :]  # [P, PPAD, WP]
                A = ttmp.tile([P, PPAD, W], f16, name="A")
                TT = ttmp.tile([P, PPAD, W], f16, name="T")
                DD = ttmp.tile([P, PPAD, W], f16, name="D")
                nc.vector.tensor_sub(A, xv[:, :, 0:W], xv[:, :, 2 : W + 2])
                nc.vector.tensor_add(TT, xv[:, :, 1 : W + 1], xv[:, :, 2 : W + 2])
                nc.vector.tensor_sub(DD, xv[:, :, 1 : W + 1], xv[:, :, 2 : W + 2])
                helpers[(p, kb)] = (A, TT, DD)

            def rhs_view(p, kb, u, lr0, kh):
                A, TT, DD = helpers[(p, kb)]
                rows = slice(lr0 + kh, lr0 + kh + CH_ROWS)
                if u == 0:
                    return A[:, rows, 0::2]
                if u == 1:
                    return TT[:, rows, 0::2]
                if u == 2:
                    return DD[:, rows, 0::2]
                return A[:, rows, 1::2]

            # ---------- winograd coefficients + transformed weights ----------
            cu = const.tile([P, NU, K], f32)
            for u in range(NU):
                for kw in range(K):
                    g = G_ROWS[u][kw]
                    if g != 0.0:
                        nc.vector.tensor_scalar_mul(
                            out=cu[:, u, kw : kw + 1],
                            in0=demod[:, kw : kw + 1],
                            scalar1=float(USCALE * g * U_SIGN[u]),
                        )

            Ub = const.tile([P, KB, K, NU, MB, P], f16)

            def emit_U(u):
                nz = [kw for kw in range(K) if G_ROWS[u][kw] != 0.0]
                for kb in range(KB):
                    for kh in range(K):
                        for mb in range(MB):
                            srcs = [
                                wfin[:, kb, kh * K + kw, mb, :] for kw in nz
                            ]
                            dst = Ub[:, kb, kh, u, mb, :]
                            if len(nz) == 1:
                                nc.scalar.activation(
                                    out=dst, in_=srcs[0], func=Act.Copy,
                                    scale=cu[:, u, nz[0] : nz[0] + 1],
                                )
                            else:
                                ta = utmp.tile([P, P], f32, name="ta")
                                nc.scalar.activation(
                                    out=ta, in_=srcs[0], func=Act.Copy,
                                    scale=cu[:, u, nz[0] : nz[0] + 1],
                                )
                                tb = utmp.tile([P, P], f32, name="tb")
                                nc.vector.scalar_tensor_tensor(
                                    out=tb, in0=srcs[1],
                                    scalar=cu[:, u, nz[1] : nz[1] + 1],
                                    in1=ta, op0=Alu.mult, op1=Alu.add,
                                )
                                nc.vector.scalar_tensor_tensor(
                                    out=dst, in0=srcs[2],
                                    scalar=cu[:, u, nz[2] : nz[2] + 1],
                                    in1=tb, op0=Alu.mult, op1=Alu.add,
                                )

            # piece-0 helpers can run on DVE while ACT/PE finish the demod
            emit_helpers(0, 0)
            emit_helpers(0, 1)
            for u in range(NU):
                emit_U(u)

            if stage == "wprep":
                ot = outp.tile([P, KB * K * NU * MB * P], f16)
                nc.vector.tensor_copy(
                    out=ot, in_=Ub[:].rearrange("p a b c d e -> p (a b c d e)")
                )
                nc.sync.dma_start(out=y_flat[0:P, 0 : KB * K * NU * MB * P], in_=ot)
                ot2 = outp.tile([P, K], f16)
                nc.vector.tensor_copy(out=ot2, in_=demod)
                nc.sync.dma_start(out=y_flat[0:P, 16000 : 16000 + K], in_=ot2)

            if stage == "full":
                # ---------- conv over 8 pieces ----------
                def emit_chunk(p, c):
                    lr0 = c * CH_ROWS
                    r0 = p * PROWS + lr0
                    yt = outp.tile([P, MB, CH_ROWS, W], f32, name="yt")
                    ots = outp.tile([P, MB, CH_ROWS, W], f16, name="ot")
                    for mb in range(MB):
                        pt = psum.tile([P, NU, CH_ROWS, NT], f32, name="pc")
                        for u in range(NU):
                            first = True
                            for kb in range(KB):
                                for kh in range(K):
                                    nc.tensor.matmul(
                                        pt[:, u],
                                        Ub[:, kb, kh, u, mb, :],
                                        rhs_view(p, kb, u, lr0, kh),
                                        start=first,
                                        stop=(kb == KB - 1 and kh == K - 1),
                                    )
                                    first = False
                        # ---- epilogue ----
                        c1 = ctmp.tile([P, CH_ROWS, NT], f32, name="c1")
                        c2 = ctmp.tile([P, CH_ROWS, NT], f32, name="c2")
                        nc.scalar.activation(out=c1, in_=pt[:, 1], func=Act.Copy)
                        nc.scalar.activation(out=c2, in_=pt[:, 2], func=Act.Copy)
                        s01 = ctmp.tile([P, CH_ROWS, NT], f32, name="s01")
                        d12 = ctmp.tile([P, CH_ROWS, NT], f32, name="d12")
                        nc.vector.tensor_add(s01, c1, c2)
                        nc.vector.tensor_sub(d12, c1, c2)
                        ytv = yt[:, mb]
                        nc.vector.tensor_add(ytv[:, :, 0::2], s01, pt[:, 0])
                        nc.vector.tensor_sub(ytv[:, :, 1::2], d12, pt[:, 3])
                        # leaky relu + 1/64 descale + f16 cast on ACT
                        nc.scalar.activation(
                            out=ots[:, mb], in_=yt[:, mb], func=Act.Prelu,
                            scale=1.0 / USCALE, alpha=0.2)
                    nc.sync.dma_start(
                        out=y_pmf[:, :, r0 * W : r0 * W + CH_ROWS * W],
                        in_=ots,
                    )

                for p in range(NP):
                    emit_chunk(p, 0)
                    if p + 1 < NP:
                        emit_helpers(p + 1, 0)
                    emit_chunk(p, 1)
                    if p + 1 < NP:
                        emit_helpers(p + 1, 1)
    nc.compile()
    return nc


def _get_nc():
    if "nc" not in _CACHE:
        _CACHE["nc"] = _build()
    return _CACHE["nc"]


def prep_in_maps(input_vector, style_vector, weight):
    """Host-side staging: fp16 casts, per-core input dicts."""
    x16 = np.ascontiguousarray(input_vector, dtype=np.float16)
    w16 = np.ascontiguousarray(weight, dtype=np.float16)
    s32 = np.ascontiguousarray(style_vector, dtype=np.float32)
    return [
        {"x": x16[b], "style": s32[b : b + 1], "w": w16}
        for b in range(B)
    ]


def _get_runner():
    """Build (once) a reusable jitted shard_map runner over the 8 cores, so
    repeated kernel() calls skip re-tracing/lowering the bass module."""
    if "runner" in _CACHE:
        return _CACHE["runner"]

    import jax
    import concourse.bass2jax as b2j
    import concourse.mybir as mybir
    from jax.experimental.shard_map import shard_map
    from jax.sharding import Mesh, PartitionSpec

    nc = _get_nc()
    b2j.install_neuronx_cc_hook()

    partition_name = nc.partition_id_tensor.name if nc.partition_id_tensor else None
    in_names, out_names, out_avals, zero_outs = [], [], [], []
    for alloc in nc.m.functions[0].allocations:
        if not isinstance(alloc, mybir.MemoryLocationSet):
            continue
        name = alloc.memorylocations[0].name
        if alloc.kind == "ExternalInput":
            if name != partition_name:
                in_names.append(name)
        elif alloc.kind == "ExternalOutput":
            out_names.append(name)
            shape = tuple(alloc.tensor_shape)
            dtype = mybir.dt.np(alloc.dtype)
            out_avals.append(jax.core.ShapedArray(shape, dtype))
            zero_outs.append(np.zeros(shape, dtype))
    n_params = len(in_names)
    n_outs = len(out_avals)
    all_in_names = list(in_names) + list(out_names)
    if partition_name is not None:
        all_in_names.append(partition_name)

    def _body(*args):
        operands = list(args)
        if partition_name is not None:
            operands.append(b2j.partition_id_tensor())
        outs = b2j._bass_exec_p.bind(
            *operands,
            out_avals=tuple(out_avals),
            in_names=tuple(all_in_names),
            out_names=tuple(out_names),
            lowering_input_output_aliases=(),
            sim_require_finite=True,
            sim_require_nnan=True,
            nc=nc,
        )
        return tuple(outs)

    devices = jax.devices()[:B]
    mesh = Mesh(np.asarray(devices), ("core",))
    in_specs = (PartitionSpec("core"),) * (n_params + n_outs)
    out_specs = (PartitionSpec("core"),) * len(out_names)
    sharded = jax.jit(
        shard_map(_body, mesh=mesh, in_specs=in_specs, out_specs=out_specs,
                  check_rep=False),
        donate_argnums=tuple(range(n_params, n_params + n_outs)),
        keep_unused=True,
    )
    _CACHE["runner"] = (sharded, in_names, out_names, out_avals, zero_outs)
    return _CACHE["runner"]


def finish_out(y_stack):
    """Raw stacked per-core outputs (B, COUT, H, W) -> full f32 output."""
    return np.ascontiguousarray(y_stack).astype(np.float32)


def kernel(input_vector, style_vector, weight):
    in_maps = prep_in_maps(input_vector, style_vector, weight)
    try:
        sharded, in_names, out_names, out_avals, zero_outs = _get_runner()
        concat_in = [
            np.concatenate([in_maps[c][nm] for c in range(B)], axis=0)
            for nm in in_names
        ]
        zeros = [
            np.zeros((B * z.shape[0], *z.shape[1:]), z.dtype) for z in zero_outs
        ]
        out_arrs = sharded(*concat_in, *zeros)
        yi = out_names.index("y")
        out = np.asarray(out_arrs[yi]).reshape(B, *out_avals[yi].shape)
    except Exception:
        # fallback: the one-shot path (slower per call, same result)
        from concourse.bass_utils import run_bass_kernel_spmd

        _CACHE.pop("runner", None)
        res = run_bass_kernel_spmd(_get_nc(), in_maps, core_ids=list(range(B)))
        out = np.stack([res.results[b]["y"] for b in range(B)], axis=0)
    return out.astype(np.float32)
